# revision 43
# baseline (speedup 1.0000x reference)
"""Trainium2 Bass kernel for nn_AspectModel (span-attention aspect tagger).

Strategy: batch-shard the 32 sentences 4-per-core across 8 NeuronCores with a
strided assignment (sentence b -> core b%8, slot l = b//8).  Lengths arrive
sorted descending, so the per-l maxima decay and the k=1 (s >= 128) half of
the transposed score memory is shipped only up to the live length per slot.
Fragments route to the core owning their sentence; per-l slot capacities are
sized to the stratum's max fragment count (both graph parameters are derived
from the inputs at compile time and cached).

The host computes the O(F*D) span featurization (l_word / word_state /
r_word gathers + short segment sums) in fp32 and ships the span features
directly as an fp8 operand, along with the span-derived scalar c and the
exact-fp32 span part of the tag logits.  The device keeps the O(F*S*D)
attention math: the v = span @ att_w projection (fp8 x fp8 DoubleRow
matmuls), the score matmuls against the pre-transposed fp8 memory, the
masked softmax chain, and the attention mix, shipped raw in fp8.  The host
applies the tiny tag projection, 1/denominator, tag_b, and log_softmax.

Scheduling notes (all measured on HW): the per-slot score bias c is folded
into the score matmul accumulation as a rank-1 ones x c-row term; the keep
mask is recomputed on-chip as (pw > 0); DMA completion semaphores fire ~2us
after the data lands under HBM load, so the stream ships each operand one
consumer ahead of its use (aw -> pw/c -> memT k0 -> memT k1 -> x k0 ->
x k1); pinned warm-up matmul bursts hold the PE clock ladder at full speed
across the softmax bubble; outputs avoid the scalar HWDGE ring (it adds ~1us
of end-of-kernel drain).
"""

import sys
import types

import ml_dtypes
import numpy as np

# Optional shim so run_bass_kernel_spmd(trace=True) works in containers where
# antenv.axon_hooks is missing (profiling only; correctness path unaffected).
try:
    import antenv.axon_hooks  # noqa: F401
except ImportError:
    try:
        from trn_agent_boot.trn_boot import _ntff_profile_via_ctypes

        _hook = _ntff_profile_via_ctypes("/opt/axon/libaxon_pjrt.so")
        _mod = types.ModuleType("antenv.axon_hooks")
        _mod.get_axon_ntff_profile_hook = lambda: _hook
        _mod.set_axon_ntff_profile_hook = lambda h: None
        sys.modules["antenv.axon_hooks"] = _mod
    except Exception:
        pass

import concourse.bass as bass  # noqa: E402
import concourse.tile as tile  # noqa: E402
from concourse import bacc, mybir  # noqa: E402
from concourse import bass_utils  # noqa: E402
from concourse.bass_utils import run_bass_kernel_spmd  # noqa: E402

# No artifact bucket in the sandbox; make tracing's upload step a no-op.
bass_utils.upload_artifacts = lambda tmpdir: f"local:{tmpdir}"

F32 = mybir.dt.float32
BF16 = mybir.dt.bfloat16
F8 = mybir.dt.float8e4
ALU = mybir.AluOpType
ACT = mybir.ActivationFunctionType

B, S, D, F, T = 32, 256, 512, 1024, 5
NCORES = 8
SEN = 4          # sentences per core
G = 44           # fragment slots per sentence (dataset max is 43)
C = SEN * G      # 176 fragment slots per core
KH = 128         # half sentence length (two k-halves per sentence)
KK = 12          # contraction chunks of the 3D=1536 span-feature dim

NPBF = ml_dtypes.bfloat16
NPF8 = ml_dtypes.float8_e4m3fn

TRACE = False
LAST_RESULT = None  # BassKernelResults of the most recent run (for test.py)

_compiled = {}


def _build(s1, gl):
    """Build + compile the per-core SPMD graph (identical on all 8 cores).

    s1[l]: live k=1 sequence length (lengths[8l] - 128) per sentence slot l --
    sentences are strided core = b % 8, l = b // 8, so the per-l maxima of the
    (descending) lengths decay and most of the k=1 memory copy is never
    shipped (scores there are fully masked).

    gl[l]: fragment-slot capacity for sentence slot l (stratum max count,
    rounded up); slot columns for l start at sum(gl[:l]).
    """
    off = np.cumsum([0] + [4 * s for s in s1]).tolist()
    M1 = off[-1]
    C = sum(gl)
    SL = np.cumsum([0] + list(gl)).tolist()
    nc = bacc.Bacc("TRN2", target_bir_lowering=False, debug=False,
                   num_devices=NCORES)

    spanB_d = nc.dram_tensor("spanB", [128, KK, C], F8, kind="ExternalInput")
    aw_d = nc.dram_tensor("aw", [128, KK, D], F8, kind="ExternalInput")
    memT0_d = nc.dram_tensor("memT0", [128, SEN, 4, KH], F8,
                             kind="ExternalInput")
    memT1_d = nc.dram_tensor("memT1", [128, max(M1, 4)], F8,
                             kind="ExternalInput")
    # pw (k=0,1) and the c row (partition 0 of slice 2) packed into one
    # tensor so the mask family costs a single DMA trigger.  The keep mask
    # kp is recomputed on-chip as (pw > 0) -- pw is exactly zero iff masked.
    pwk_d = nc.dram_tensor("pwk", [128, 3, C], BF16, kind="ExternalInput")
    x_d = nc.dram_tensor("x", [128, 2, SEN, D], F8, kind="ExternalInput")
    out2_d = nc.dram_tensor("out2", [1, C], F32, kind="ExternalOutput")
    # raw attention mix, [slot-packed partitions, l-pair, d] fp8
    out3_d = nc.dram_tensor("out3", [128, 2, D], F8, kind="ExternalOutput")

    with tile.TileContext(nc) as tc:
        with (
            tc.tile_pool(name="persist", bufs=1) as pp,
            tc.tile_pool(name="work", bufs=2) as wp,
            tc.tile_pool(name="psum", bufs=2, space="PSUM") as psp,
        ):
            # ---- persistent SBUF tensors ----
            spanB = pp.tile([128, KK, C], F8, tag="spanB")
            aw_sb = pp.tile([128, KK, D], F8, tag="aw_sb")
            memT0 = pp.tile([128, SEN, 4, KH], F8, tag="memT0")
            memT1 = pp.tile([128, max(M1, 4)], F8, tag="memT1")
            pwk = pp.tile([128, 3, C], BF16, tag="pwk")
            kp_sb = pp.tile([128, 2, C], BF16, tag="kp_sb")
            x_sb = pp.tile([128, 2, SEN, D], F8, tag="x_sb")
            v_sb = pp.tile([128, 4, C], F8, tag="v_sb")
            wTu = pp.tile([128, 2, C], BF16, tag="wTu")
            uT = pp.tile([128, 2, C], BF16, tag="uT")
            mix_sb = pp.tile([128, 2, D], F8, tag="mix_sb")
            out2_sb = pp.tile([1, C], F32, tag="out2_sb")
            ones1 = pp.tile([128, 1], BF16, tag="ones1")
            onesr = pp.tile([1, 128], BF16, tag="onesr")
            warm = pp.tile([128, 256], BF16, tag="warm")

            # ---- input DMAs: one sync queue, ordered by first use ----
            # Nine triggers (each costs ~650ns of sync-queue time, so fewer,
            # larger DMAs keep the stream data-bound).  Arrival order matches
            # need order: the k=0 chain runs first, k=1 second; DMA completion
            # semaphores fire ~2us after the data under load, so each
            # consumer needs its operand shipped one slot ahead.
            prev = None
            bulk = [(spanB[:], spanB_d.ap())]
            bulk += [(aw_sb[:, 4 * g:4 * g + 4], aw_d.ap()[:, 4 * g:4 * g + 4])
                     for g in range(3)]
            bulk += [(pwk[:], pwk_d.ap()),
                     (memT0[:], memT0_d.ap()),
                     (memT1[:], memT1_d.ap()),
                     (x_sb[:, 0:1], x_d.ap()[:, 0:1]),
                     (x_sb[:, 1:2], x_d.ap()[:, 1:2])]
            for dst, src in bulk:
                d = nc.sync.dma_start(dst, src)
                if prev is not None:
                    tile.add_dep_helper(d.ins, prev.ins, sync=False,
                                        reason="bulk ring order")
                prev = d

            # ---- constants ----
            nc.gpsimd.memset(ones1[:], 1.0)
            nc.gpsimd.memset(onesr[:], 1.0)
            nc.gpsimd.memset(warm[:], 0.0)

            evac = [nc.vector.tensor_copy,
                    lambda dst, src: nc.scalar.copy(dst, src)]

            # keep mask from pw: masked positions have pw == 0 exactly
            nc.vector.tensor_scalar(out=kp_sb[:], in0=pwk[:, 0:2],
                                    scalar1=0.0, scalar2=None,
                                    op0=ALU.is_gt)

            # PSUM banks: pv holds the 4 vmm accumulators (one per dj:
            # interleaved accumulation groups must not share a bank) and later
            # the two mix tiles; gt holds the two score grids; pd the
            # denominator row; warmp is a dedicated bank for p-state warm-up
            # matmuls so they carry no false deps on live banks.
            pv = [psp.tile([128, 512], F32, tag="pv", name=f"pv{dj}", bufs=4)
                  for dj in range(4)]
            warmp = psp.tile([128, 256], F32, tag="warmp", bufs=1)

            def warm_mm(n, after=None):
                # `after` pins the burst's position in the PE queue (the Tile
                # scheduler otherwise hoists dep-free warm matmuls earlier).
                prev_w = after
                for _ in range(n):
                    w = nc.tensor.matmul(warmp[:], warm[:, 0:128], warm[:],
                                         start=True, stop=True)
                    if prev_w is not None:
                        tile.add_dep_helper(w.ins, prev_w.ins, sync=False,
                                            reason="warm order")
                    prev_w = w
                return prev_w

            # ---- PE warm-up: keep the tensor engine busy while the first
            # inputs stream (the clock ladder needs continuous activity).
            # The PE queue is in-order, so the burst is sized to end right
            # when vmm's first operands land -- more warms would delay vmm.
            warm_mm(12)

            # ---- vmm: v = spanB.T @ att_w, fp8 x fp8, 12 contraction chunks
            # processed as 6 DoubleRow pairs (2 fp8 weights per PE cell ->
            # half the matmul count; operands are 3D APs [128, 2, free]).
            sc = nc.named_scope("vmm"); sc.__enter__()
            last_vmm = None
            for kk in range(0, KK, 2):
                for dj in range(4):
                    last_vmm = nc.tensor.matmul(
                        pv[dj][:, 0:C],
                        aw_sb[:, kk:kk + 2, dj * 128:(dj + 1) * 128],
                        spanB[:, kk:kk + 2, :],
                        start=(kk == 0), stop=(kk == KK - 2),
                        perf_mode=mybir.MatmulPerfMode.DoubleRow)
            for dj in range(4):
                evac[dj % 2](v_sb[:, dj, :], pv[dj][:, 0:C])
            sc.__exit__(None, None, None)

            # bridge the v-evacuation bubble so the PE stays at full clock
            # (pinned after vmm; unpinned warms get hoisted before it and
            # delay vmm's start in the in-order PE queue)
            warm_mm(6, after=last_vmm)

            # ---- scores + masked softmax chain, per k-half ----
            # gt[s, slot] = c[slot] (rank-1 init) + sum_d memT[d, s] v[d, slot]
            sc = nc.named_scope("gts"); sc.__enter__()
            gt = [psp.tile([128, C], F32, tag="gt", name=f"gt{k}", bufs=2)
                  for k in range(2)]
            chain_prev = None  # pins scalar-queue order: tanh0,exp0,tanh1,exp1
            last_gts = None
            for k in (0, 1):
                nc.tensor.matmul(gt[k][:], onesr[:], pwk[0:1, 2, :],
                                 start=True, stop=False, skip_group_check=True)
                for l in range(SEN):
                    if k == 1 and s1[l] == 0:
                        continue
                    for dj in range(4):
                        stat = (memT0[:, l, dj, :] if k == 0 else
                                memT1[:, off[l] + dj * s1[l]:
                                      off[l] + (dj + 1) * s1[l]])
                        rows = KH if k == 0 else s1[l]
                        last_gts = nc.tensor.matmul(
                            gt[k][0:rows, SL[l]:SL[l + 1]],
                            stat,
                            v_sb[:, dj, SL[l]:SL[l + 1]],
                            start=False, stop=(dj == 3),
                            skip_group_check=True)
                sg = wp.tile([128, C], BF16, tag="sg", name=f"sg{k}")
                th = wp.tile([128, C], BF16, tag="th", name=f"th{k}")
                e0 = wp.tile([128, C], BF16, tag="e0", name=f"e0{k}")
                nc.vector.tensor_tensor(sg[:], gt[k][:], pwk[:, k],
                                        op=ALU.mult)
                i_th = nc.scalar.activation(th[:], sg[:], ACT.Tanh)
                if chain_prev is not None:
                    tile.add_dep_helper(i_th.ins, chain_prev.ins, sync=False,
                                        reason="scalar chain order")
                i_e0 = nc.scalar.activation(e0[:], th[:], ACT.Exp)
                chain_prev = i_e0
                nc.vector.tensor_tensor(wTu[:, k], e0[:], pwk[:, k],
                                        op=ALU.mult)
                nc.gpsimd.tensor_tensor(uT[:, k], e0[:], kp_sb[:, k],
                                        op=ALU.mult)
            sc.__exit__(None, None, None)

            # bridge the softmax-chain bubble (PE idle >2us drops the clock);
            # pinned after the score matmuls so the scheduler can't hoist it.
            last_warm = warm_mm(12, after=last_gts)

            # ---- mix[slot, d] = sum_{s,k} wTu[s, k, slot] x[s, k, d] ----
            # k-major: only the last four matmuls gate on the final x DMA.
            # sentence l lands at partitions (l%2)*64 .. +44 of tile l//2.
            sc = nc.named_scope("mix"); sc.__enter__()
            pmix = [psp.tile([128, 512], F32, tag="pv", name=f"pmix{t}",
                             bufs=4) for t in range(2)]
            for k in (0, 1):
                for l in range(SEN):
                    mmi = nc.tensor.matmul(
                        pmix[l // 2][(l % 2) * 64:(l % 2) * 64 + gl[l], :],
                        wTu[:, k, SL[l]:SL[l + 1]],
                        x_sb[:, k, l, :],
                        start=(k == 0), stop=(k == 1))
                    if k == 0 and l == 0:
                        tile.add_dep_helper(mmi.ins, last_warm.ins,
                                            sync=False, reason="after warms")
                    last_mix = mmi
            # softmax denominator (ships to host; host divides).  Pinned
            # after the mix matmuls: pd waits on the slow gpsimd uT products
            # and must not block the mix blocks in the in-order PE queue.
            pd = psp.tile([1, C], F32, tag="pd", name="pd", bufs=1)
            pd1 = nc.tensor.matmul(pd[:], ones1[:], uT[:, 0, :],
                                   start=True, stop=False)
            tile.add_dep_helper(pd1.ins, last_mix.ins, sync=False,
                                reason="pd after mix")
            nc.tensor.matmul(pd[:], ones1[:], uT[:, 1, :],
                             start=False, stop=True)
            # out2's DMA trigger goes on gpsimd so it runs parallel to the
            # mix-evac sync triggers (gpsimd can't read PSUM, so the copy
            # itself stays on vector); its ring-drain cost overlaps the out3
            # data, unlike a serial third trigger on the sync ring.
            nc.vector.tensor_copy(out2_sb[:], pd[:])
            nc.gpsimd.dma_start(out2_d.ap(), out2_sb[:])
            for t in range(2):
                for h in range(2):
                    evac[h](mix_sb[:, t, h * 256:(h + 1) * 256],
                            pmix[t][:, h * 256:(h + 1) * 256])
                nc.sync.dma_start(out3_d.ap()[:, t, :], mix_sb[:, t, :])
            sc.__exit__(None, None, None)

    nc.compile()
    return nc


def _host_prep(en_output, lengths, frag_b, frag_s, frag_e, att_w, att_b,
               tag_w, s1, gl):
    """Shard + relayout inputs.  Returns (in_maps, assign, overflow, pls)."""
    seq = float(lengths[0])
    off = np.cumsum([0] + [4 * s for s in s1]).tolist()
    M1 = off[-1]
    C = sum(gl)
    SL = np.cumsum([0] + list(gl)).tolist()

    # span features in fp64/fp32 on host: [F, 3D]
    cs = np.zeros((B, S + 1, D), np.float64)
    np.cumsum(en_output, axis=1, dtype=np.float64, out=cs[:, 1:])
    ws = (cs[frag_b, frag_e] - cs[frag_b, frag_s]).astype(np.float32)
    span = np.concatenate(
        [en_output[frag_b, frag_s], ws, en_output[frag_b, frag_e - 1]],
        axis=1)                                          # [F, 1536]
    pls = span @ tag_w[:, :3 * D].T                      # [F, T] exact fp32
    cvals = span @ att_b                                 # [F]

    aw_np = np.ascontiguousarray(
        att_w.reshape(KK, 128, D).transpose(1, 0, 2)).astype(NPF8)

    assign = np.full((F, 2), -1, dtype=np.int64)  # (core, slot) per fragment
    counts = np.zeros((NCORES, SEN), dtype=np.int64)
    overflow = []
    fs_slot = np.zeros((NCORES, C), np.float32)
    fm_slot = np.full((NCORES, C), -1.0, np.float32)
    ln_slot = np.full((NCORES, C), float(S), np.float32)
    span_slot = np.zeros((NCORES, C, 3 * D), np.float32)
    c_slot = np.zeros((NCORES, C), np.float32)

    for i in range(F):
        b = int(frag_b[i])
        core, l = b % NCORES, b // NCORES
        k = counts[core, l]
        if k >= gl[l]:
            overflow.append(i)
            continue
        counts[core, l] += 1
        slot = SL[l] + k
        assign[i] = (core, slot)
        fs_slot[core, slot] = frag_s[i]
        fm_slot[core, slot] = frag_e[i] - 1
        ln_slot[core, slot] = lengths[b]
        span_slot[core, slot] = span[i]
        c_slot[core, slot] = cvals[i]

    in_maps = []
    p = np.arange(128, dtype=np.float32)[:, None]        # [128, 1]
    for core in range(NCORES):
        fs = fs_slot[core][None, :]
        fm = fm_slot[core][None, :]
        ln = ln_slot[core][None, :]
        pwk = np.zeros((128, 3, C), np.float32)
        for k in range(2):
            s = k * KH + p
            keep = (~((s >= fs) & (s <= fm))) & (s < ln)
            dis = np.where(s < fs, fs - s, np.where(s > fm, s - fm, seq))
            pwk[:, k, :] = (1.0 - dis / seq) * keep
        pwk[0, 2, :] = c_slot[core]
        spanB = np.ascontiguousarray(
            span_slot[core].reshape(C, KK, 128).transpose(2, 1, 0)
        ).astype(NPF8)
        xs = en_output[core::NCORES]                     # [4, 256, 512]
        x_np = np.ascontiguousarray(
            xs.reshape(SEN, 2, KH, D).transpose(2, 1, 0, 3)).astype(NPF8)
        # memT[p, l, dj, s] = xs[l, s_glob, dj*128 + p]
        mT = xs.reshape(SEN, 2, KH, 4, 128).transpose(4, 1, 0, 3, 2)
        memT0_np = np.ascontiguousarray(mT[:, 0]).astype(NPF8)
        memT1_np = np.zeros((128, max(M1, 4)), np.float32)
        for l in range(SEN):
            if s1[l]:
                memT1_np[:, off[l]:off[l + 1]] = \
                    mT[:, 1, l, :, :s1[l]].reshape(128, 4 * s1[l])
        in_maps.append({
            "spanB": spanB, "aw": aw_np,
            "memT0": memT0_np, "memT1": memT1_np.astype(NPF8),
            "pwk": np.ascontiguousarray(pwk).astype(NPBF),
            "x": x_np,
        })
    return in_maps, assign, overflow, pls


def _host_fragment(en_output, lengths, s, e, b, att_w, att_b, tag_w, tag_b,
                   seq_len):
    """Numpy fallback for (vanishingly rare) slot-overflow fragments."""
    mem = en_output[b].astype(np.float64)
    ws = mem[s:e].sum(0)
    span = np.concatenate([mem[s], ws, mem[e - 1]])
    pos = np.arange(S)
    in_span = (pos >= s) & (pos < e)
    att_mask = in_span | (pos >= lengths[b])
    dis = np.where(pos < s, s - pos,
                   np.where(pos >= e, pos - e + 1, seq_len)).astype(np.float64)
    pwv = 1.0 - dis / seq_len
    fin = pwv[:, None] * mem
    v = span @ att_w.astype(np.float64)
    c = span @ att_b.astype(np.float64)
    sc = np.tanh(fin @ v + c)
    sc = np.where(att_mask, -1e4, sc)
    sc = sc - sc.max()
    a = np.exp(sc)
    a = a / a.sum()
    mix = a @ fin
    ms = np.concatenate([span, mix])
    lg = ms @ tag_w.astype(np.float64).T + tag_b.astype(np.float64)
    lg = lg - lg.max()
    return (lg - np.log(np.exp(lg).sum())).astype(np.float32)


def kernel(en_output, lengths, frag_b, frag_s, frag_e, att_w, att_b, tag_w,
           tag_b):
    global LAST_RESULT
    en_output = np.asarray(en_output, dtype=np.float32)
    lengths = np.asarray(lengths).astype(np.int64)
    frag_b = np.asarray(frag_b).astype(np.int64)
    frag_s = np.asarray(frag_s).astype(np.int64)
    frag_e = np.asarray(frag_e).astype(np.int64)
    att_w = np.asarray(att_w, dtype=np.float32)
    att_b = np.asarray(att_b, dtype=np.float32)
    tag_w = np.asarray(tag_w, dtype=np.float32)
    tag_b = np.asarray(tag_b, dtype=np.float32)

    # live k=1 length per sentence slot l: max over cores of the strided
    # assignment (sentence b -> core b % 8, slot l = b // 8), rounded up
    s1 = tuple(int(min(KH, max(0, (max(lengths[8 * l:8 * l + 8]) - KH + 7)
                               // 8 * 8))) for l in range(SEN))
    # per-l fragment-slot capacity: stratum max count, rounded up to 4,
    # keeping the total 16-aligned (DoubleRow moving-AP stride constraint)
    cnt = np.bincount(frag_b, minlength=B)
    gl = [max(1, int(max(cnt[8 * l:8 * l + 8]) + 3) // 4 * 4)
          for l in range(SEN)]
    gl[-1] += (-sum(gl)) % 16
    gl = tuple(gl)
    key = ("nc", s1, gl)
    if key not in _compiled:
        _compiled[key] = _build(s1, gl)
    nc = _compiled[key]

    in_maps, assign, overflow, pls = _host_prep(
        en_output, lengths, frag_b, frag_s, frag_e, att_w, att_b, tag_w,
        s1, gl)

    res = run_bass_kernel_spmd(nc, in_maps, core_ids=list(range(NCORES)),
                               trace=TRACE)
    LAST_RESULT = res

    C = sum(gl)
    SL = np.cumsum([0] + list(gl)).tolist()
    tw_mix = tag_w[:, 3 * D:]                            # [T, D]
    out = np.empty((F, T), dtype=np.float32)
    per_core = []
    for i in range(NCORES):
        dn = np.asarray(res.results[i]["out2"], np.float32)[0]  # [C]
        mx = np.asarray(res.results[i]["out3"], np.float32)     # [128, 2, D]
        mix = np.empty((C, D), np.float32)
        for l in range(SEN):
            mix[SL[l]:SL[l + 1]] = \
                mx[(l % 2) * 64:(l % 2) * 64 + gl[l], l // 2]
        per_core.append((mix @ tw_mix.T) / dn[:, None])         # [C, T]
    cores = assign[:, 0]
    slots = assign[:, 1]
    lg = np.empty((F, T), np.float32)
    for core in range(NCORES):
        sel = cores == core
        lg[sel] = per_core[core][slots[sel]]
    lg += pls + tag_b[None, :]
    m = lg.max(axis=1, keepdims=True)
    ls = lg - m
    out = ls - np.log(np.exp(ls).sum(axis=1, keepdims=True))
    seq_len = float(lengths[0])
    for i in overflow:
        out[i] = _host_fragment(en_output, lengths, int(frag_s[i]),
                                int(frag_e[i]), int(frag_b[i]), att_w, att_b,
                                tag_w, tag_b, seq_len)
    return out


# revision 44
# speedup vs baseline: 1.0141x; 1.0141x over previous
"""Trainium2 Bass kernel for nn_AspectModel (span-attention aspect tagger).

Strategy: batch-shard the 32 sentences 4-per-core across 8 NeuronCores with a
strided assignment (sentence b -> core b%8, slot l = b//8).  Lengths arrive
sorted descending, so the per-l maxima decay and the k=1 (s >= 128) half of
the transposed score memory is shipped only up to the live length per slot.
Fragments route to the core owning their sentence; per-l slot capacities are
sized to the stratum's max fragment count (both graph parameters are derived
from the inputs at compile time and cached).

The host computes the O(F*D) span featurization (l_word / word_state /
r_word gathers + short segment sums) in fp32 and ships the span features
directly as an fp8 operand, along with the span-derived scalar c and the
exact-fp32 span part of the tag logits.  The device keeps the O(F*S*D)
attention math: the v = span @ att_w projection (fp8 x fp8 DoubleRow
matmuls), the score matmuls against the pre-transposed fp8 memory, the
masked softmax chain, and the attention mix, shipped raw in fp8.  The host
applies the tiny tag projection, 1/denominator, tag_b, and log_softmax.

Scheduling notes (all measured on HW): the per-slot score bias c is folded
into the score matmul accumulation as a rank-1 ones x c-row term; the keep
mask is recomputed on-chip as (pw > 0); DMA completion semaphores fire ~2us
after the data lands under HBM load, so the stream ships each operand one
consumer ahead of its use (aw -> pw/c -> memT k0 -> memT k1 -> x k0 ->
x k1); pinned warm-up matmul bursts hold the PE clock ladder at full speed
across the softmax bubble; outputs avoid the scalar HWDGE ring (it adds ~1us
of end-of-kernel drain).
"""

import sys
import types

import ml_dtypes
import numpy as np

# Optional shim so run_bass_kernel_spmd(trace=True) works in containers where
# antenv.axon_hooks is missing (profiling only; correctness path unaffected).
try:
    import antenv.axon_hooks  # noqa: F401
except ImportError:
    try:
        from trn_agent_boot.trn_boot import _ntff_profile_via_ctypes

        _hook = _ntff_profile_via_ctypes("/opt/axon/libaxon_pjrt.so")
        _mod = types.ModuleType("antenv.axon_hooks")
        _mod.get_axon_ntff_profile_hook = lambda: _hook
        _mod.set_axon_ntff_profile_hook = lambda h: None
        sys.modules["antenv.axon_hooks"] = _mod
    except Exception:
        pass

import concourse.bass as bass  # noqa: E402
import concourse.tile as tile  # noqa: E402
from concourse import bacc, mybir  # noqa: E402
from concourse import bass_utils  # noqa: E402
from concourse.bass_utils import run_bass_kernel_spmd  # noqa: E402

# No artifact bucket in the sandbox; make tracing's upload step a no-op.
bass_utils.upload_artifacts = lambda tmpdir: f"local:{tmpdir}"

F32 = mybir.dt.float32
BF16 = mybir.dt.bfloat16
F8 = mybir.dt.float8e4
ALU = mybir.AluOpType
ACT = mybir.ActivationFunctionType

B, S, D, F, T = 32, 256, 512, 1024, 5
NCORES = 8
SEN = 4          # sentences per core
G = 44           # fragment slots per sentence (dataset max is 43)
C = SEN * G      # 176 fragment slots per core
KH = 128         # half sentence length (two k-halves per sentence)
KK = 12          # contraction chunks of the 3D=1536 span-feature dim

NPBF = ml_dtypes.bfloat16
NPF8 = ml_dtypes.float8_e4m3fn

TRACE = False
LAST_RESULT = None  # BassKernelResults of the most recent run (for test.py)

_compiled = {}


def _build(s1, gl):
    """Build + compile the per-core SPMD graph (identical on all 8 cores).

    s1[l]: live k=1 sequence length (lengths[8l] - 128) per sentence slot l --
    sentences are strided core = b % 8, l = b // 8, so the per-l maxima of the
    (descending) lengths decay and most of the k=1 memory copy is never
    shipped (scores there are fully masked).

    gl[l]: fragment-slot capacity for sentence slot l (stratum max count,
    rounded up); slot columns for l start at sum(gl[:l]).
    """
    off = np.cumsum([0] + [4 * s for s in s1]).tolist()
    M1 = off[-1]
    C = sum(gl)
    SL = np.cumsum([0] + list(gl)).tolist()
    nc = bacc.Bacc("TRN2", target_bir_lowering=False, debug=False,
                   num_devices=NCORES)

    spanB_d = nc.dram_tensor("spanB", [128, KK, C], F8, kind="ExternalInput")
    aw_d = nc.dram_tensor("aw", [128, KK, D], F8, kind="ExternalInput")
    memT0_d = nc.dram_tensor("memT0", [128, SEN, 4, KH], F8,
                             kind="ExternalInput")
    memT1_d = nc.dram_tensor("memT1", [128, max(M1, 4)], F8,
                             kind="ExternalInput")
    # pw (k=0,1) and the c row (partition 0 of slice 2) packed into one
    # tensor so the mask family costs a single DMA trigger.  The keep mask
    # kp is recomputed on-chip as (pw > 0) -- pw is exactly zero iff masked.
    pwk_d = nc.dram_tensor("pwk", [128, 3, C], BF16, kind="ExternalInput")
    x_d = nc.dram_tensor("x", [128, 2, SEN, D], F8, kind="ExternalInput")
    out2_d = nc.dram_tensor("out2", [1, C], F32, kind="ExternalOutput")
    # raw attention mix, [slot-packed partitions, l-pair, d] fp8
    out3_d = nc.dram_tensor("out3", [128, 2, D], F8, kind="ExternalOutput")

    with tile.TileContext(nc) as tc:
        with (
            tc.tile_pool(name="persist", bufs=1) as pp,
            tc.tile_pool(name="work", bufs=2) as wp,
            tc.tile_pool(name="psum", bufs=2, space="PSUM") as psp,
        ):
            # ---- persistent SBUF tensors ----
            spanB = pp.tile([128, KK, C], F8, tag="spanB")
            aw_sb = pp.tile([128, KK, D], F8, tag="aw_sb")
            memT0 = pp.tile([128, SEN, 4, KH], F8, tag="memT0")
            memT1 = pp.tile([128, max(M1, 4)], F8, tag="memT1")
            pwk = pp.tile([128, 3, C], BF16, tag="pwk")
            kp_sb = pp.tile([128, 2, C], BF16, tag="kp_sb")
            x_sb = pp.tile([128, 2, SEN, D], F8, tag="x_sb")
            v_sb = pp.tile([128, 4, C], F8, tag="v_sb")
            wTu = pp.tile([128, 2, C], BF16, tag="wTu")
            uT = pp.tile([128, 2, C], BF16, tag="uT")
            mix_sb = pp.tile([128, 2, D], F8, tag="mix_sb")
            out2_sb = pp.tile([1, C], F32, tag="out2_sb")
            ones1 = pp.tile([128, 1], BF16, tag="ones1")
            onesr = pp.tile([1, 128], BF16, tag="onesr")
            warm = pp.tile([128, 256], BF16, tag="warm")

            # ---- input DMAs: one sync queue, ordered by first use ----
            # Nine triggers (each costs ~650ns of sync-queue time, so fewer,
            # larger DMAs keep the stream data-bound).  Arrival order matches
            # need order: the k=0 chain runs first, k=1 second; DMA completion
            # semaphores fire ~2us after the data under load, so each
            # consumer needs its operand shipped one slot ahead.
            prev = None
            bulk = [(spanB[:], spanB_d.ap())]
            bulk += [(aw_sb[:, 4 * g:4 * g + 4], aw_d.ap()[:, 4 * g:4 * g + 4])
                     for g in range(3)]
            bulk += [(pwk[:], pwk_d.ap()),
                     (memT0[:], memT0_d.ap()),
                     (memT1[:], memT1_d.ap()),
                     (x_sb[:, 0:1], x_d.ap()[:, 0:1]),
                     (x_sb[:, 1:2], x_d.ap()[:, 1:2])]
            for dst, src in bulk:
                d = nc.sync.dma_start(dst, src)
                if prev is not None:
                    tile.add_dep_helper(d.ins, prev.ins, sync=False,
                                        reason="bulk ring order")
                prev = d

            # ---- constants ----
            nc.gpsimd.memset(ones1[:], 1.0)
            nc.gpsimd.memset(onesr[:], 1.0)
            nc.gpsimd.memset(warm[:], 0.0)

            evac = [nc.vector.tensor_copy,
                    lambda dst, src: nc.scalar.copy(dst, src)]

            # keep mask from pw: masked positions have pw == 0 exactly
            nc.vector.tensor_scalar(out=kp_sb[:], in0=pwk[:, 0:2],
                                    scalar1=0.0, scalar2=None,
                                    op0=ALU.is_gt)

            # PSUM banks: pv holds the 4 vmm accumulators (one per dj:
            # interleaved accumulation groups must not share a bank) and later
            # the two mix tiles; gt holds the two score grids; pd the
            # denominator row; warmp is a dedicated bank for p-state warm-up
            # matmuls so they carry no false deps on live banks.
            pv = [psp.tile([128, 512], F32, tag="pv", name=f"pv{dj}", bufs=4)
                  for dj in range(4)]
            warmp = psp.tile([128, 256], F32, tag="warmp", bufs=1)

            def warm_mm(n, after=None):
                # `after` pins the burst's position in the PE queue (the Tile
                # scheduler otherwise hoists dep-free warm matmuls earlier).
                prev_w = after
                for _ in range(n):
                    w = nc.tensor.matmul(warmp[:], warm[:, 0:128], warm[:],
                                         start=True, stop=True)
                    if prev_w is not None:
                        tile.add_dep_helper(w.ins, prev_w.ins, sync=False,
                                            reason="warm order")
                    prev_w = w
                return prev_w

            # ---- PE warm-up: keep the tensor engine busy while the first
            # inputs stream (the clock ladder needs continuous activity).
            # The PE queue is in-order, so the burst is sized to end right
            # when vmm's first operands land -- more warms would delay vmm.
            warm_mm(12)

            # ---- vmm: v = spanB.T @ att_w, fp8 x fp8, 12 contraction chunks
            # processed as 6 DoubleRow pairs (2 fp8 weights per PE cell ->
            # half the matmul count; operands are 3D APs [128, 2, free]).
            sc = nc.named_scope("vmm"); sc.__enter__()
            last_vmm = None
            for kk in range(0, KK, 2):
                for dj in range(4):
                    last_vmm = nc.tensor.matmul(
                        pv[dj][:, 0:C],
                        aw_sb[:, kk:kk + 2, dj * 128:(dj + 1) * 128],
                        spanB[:, kk:kk + 2, :],
                        start=(kk == 0), stop=(kk == KK - 2),
                        perf_mode=mybir.MatmulPerfMode.DoubleRow)
            for dj in range(4):
                evac[dj % 2](v_sb[:, dj, :], pv[dj][:, 0:C])
            sc.__exit__(None, None, None)

            # bridge the v-evacuation bubble so the PE stays at full clock
            # (pinned after vmm; unpinned warms get hoisted before it and
            # delay vmm's start in the in-order PE queue)
            warm_mm(6, after=last_vmm)

            # ---- scores + masked softmax chain, per k-half ----
            # gt[s, slot] = c[slot] (rank-1 init) + sum_d memT[d, s] v[d, slot]
            sc = nc.named_scope("gts"); sc.__enter__()
            gt = [psp.tile([128, C], F32, tag="gt", name=f"gt{k}", bufs=2)
                  for k in range(2)]
            chain_prev = None  # pins scalar-queue order: tanh0,exp0,tanh1,exp1
            last_gts = None
            for k in (0, 1):
                nc.tensor.matmul(gt[k][:], onesr[:], pwk[0:1, 2, :],
                                 start=True, stop=False, skip_group_check=True)
                for l in range(SEN):
                    if k == 1 and s1[l] == 0:
                        continue
                    for dj in range(4):
                        stat = (memT0[:, l, dj, :] if k == 0 else
                                memT1[:, off[l] + dj * s1[l]:
                                      off[l] + (dj + 1) * s1[l]])
                        rows = KH if k == 0 else s1[l]
                        last_gts = nc.tensor.matmul(
                            gt[k][0:rows, SL[l]:SL[l + 1]],
                            stat,
                            v_sb[:, dj, SL[l]:SL[l + 1]],
                            start=False, stop=(dj == 3),
                            skip_group_check=True)
                sg = wp.tile([128, C], BF16, tag="sg", name=f"sg{k}")
                th = wp.tile([128, C], BF16, tag="th", name=f"th{k}")
                e0 = wp.tile([128, C], BF16, tag="e0", name=f"e0{k}")
                nc.vector.tensor_tensor(sg[:], gt[k][:], pwk[:, k],
                                        op=ALU.mult)
                i_th = nc.scalar.activation(th[:], sg[:], ACT.Tanh)
                if chain_prev is not None:
                    tile.add_dep_helper(i_th.ins, chain_prev.ins, sync=False,
                                        reason="scalar chain order")
                i_e0 = nc.scalar.activation(e0[:], th[:], ACT.Exp)
                chain_prev = i_e0
                nc.vector.tensor_tensor(wTu[:, k], e0[:], pwk[:, k],
                                        op=ALU.mult)
                nc.gpsimd.tensor_tensor(uT[:, k], e0[:], kp_sb[:, k],
                                        op=ALU.mult)
            sc.__exit__(None, None, None)

            # bridge the softmax-chain bubble (PE idle >2us drops the clock);
            # pinned after the score matmuls so the scheduler can't hoist it.
            last_warm = warm_mm(12, after=last_gts)

            # ---- mix[slot, d] = sum_{s,k} wTu[s, k, slot] x[s, k, d] ----
            # k-major: only the last four matmuls gate on the final x DMA.
            # sentence l lands at partitions (l%2)*64 .. +44 of tile l//2.
            sc = nc.named_scope("mix"); sc.__enter__()
            pmix = [psp.tile([128, 512], F32, tag="pv", name=f"pmix{t}",
                             bufs=4) for t in range(2)]
            for k in (0, 1):
                for l in range(SEN):
                    mmi = nc.tensor.matmul(
                        pmix[l // 2][(l % 2) * 64:(l % 2) * 64 + gl[l], :],
                        wTu[:, k, SL[l]:SL[l + 1]],
                        x_sb[:, k, l, :],
                        start=(k == 0), stop=(k == 1))
                    if k == 0 and l == 0:
                        tile.add_dep_helper(mmi.ins, last_warm.ins,
                                            sync=False, reason="after warms")
                    last_mix = mmi
            # softmax denominator (ships to host; host divides).  Pinned
            # after the mix matmuls: pd waits on the slow gpsimd uT products
            # and must not block the mix blocks in the in-order PE queue.
            pd = psp.tile([1, C], F32, tag="pd", name="pd", bufs=1)
            pd1 = nc.tensor.matmul(pd[:], ones1[:], uT[:, 0, :],
                                   start=True, stop=False)
            tile.add_dep_helper(pd1.ins, last_mix.ins, sync=False,
                                reason="pd after mix")
            nc.tensor.matmul(pd[:], ones1[:], uT[:, 1, :],
                             start=False, stop=True)
            # out2's DMA trigger goes on gpsimd so it runs parallel to the
            # mix-evac sync triggers (gpsimd can't read PSUM, so the copy
            # itself stays on vector); its ring-drain cost overlaps the out3
            # data, unlike a serial third trigger on the sync ring.
            nc.vector.tensor_copy(out2_sb[:], pd[:])
            nc.gpsimd.dma_start(out2_d.ap(), out2_sb[:])
            for t in range(2):
                for h in range(4):
                    evac[h % 2](mix_sb[:, t, h * 128:(h + 1) * 128],
                                pmix[t][:, h * 128:(h + 1) * 128])
                nc.sync.dma_start(out3_d.ap()[:, t, :], mix_sb[:, t, :])
            sc.__exit__(None, None, None)

    nc.compile()
    return nc


def _host_prep(en_output, lengths, frag_b, frag_s, frag_e, att_w, att_b,
               tag_w, s1, gl):
    """Shard + relayout inputs.  Returns (in_maps, assign, overflow, pls)."""
    seq = float(lengths[0])
    off = np.cumsum([0] + [4 * s for s in s1]).tolist()
    M1 = off[-1]
    C = sum(gl)
    SL = np.cumsum([0] + list(gl)).tolist()

    # span features in fp64/fp32 on host: [F, 3D]
    cs = np.zeros((B, S + 1, D), np.float64)
    np.cumsum(en_output, axis=1, dtype=np.float64, out=cs[:, 1:])
    ws = (cs[frag_b, frag_e] - cs[frag_b, frag_s]).astype(np.float32)
    span = np.concatenate(
        [en_output[frag_b, frag_s], ws, en_output[frag_b, frag_e - 1]],
        axis=1)                                          # [F, 1536]
    pls = span @ tag_w[:, :3 * D].T                      # [F, T] exact fp32
    cvals = span @ att_b                                 # [F]

    aw_np = np.ascontiguousarray(
        att_w.reshape(KK, 128, D).transpose(1, 0, 2)).astype(NPF8)

    assign = np.full((F, 2), -1, dtype=np.int64)  # (core, slot) per fragment
    counts = np.zeros((NCORES, SEN), dtype=np.int64)
    overflow = []
    fs_slot = np.zeros((NCORES, C), np.float32)
    fm_slot = np.full((NCORES, C), -1.0, np.float32)
    ln_slot = np.full((NCORES, C), float(S), np.float32)
    span_slot = np.zeros((NCORES, C, 3 * D), np.float32)
    c_slot = np.zeros((NCORES, C), np.float32)

    for i in range(F):
        b = int(frag_b[i])
        core, l = b % NCORES, b // NCORES
        k = counts[core, l]
        if k >= gl[l]:
            overflow.append(i)
            continue
        counts[core, l] += 1
        slot = SL[l] + k
        assign[i] = (core, slot)
        fs_slot[core, slot] = frag_s[i]
        fm_slot[core, slot] = frag_e[i] - 1
        ln_slot[core, slot] = lengths[b]
        span_slot[core, slot] = span[i]
        c_slot[core, slot] = cvals[i]

    in_maps = []
    p = np.arange(128, dtype=np.float32)[:, None]        # [128, 1]
    for core in range(NCORES):
        fs = fs_slot[core][None, :]
        fm = fm_slot[core][None, :]
        ln = ln_slot[core][None, :]
        pwk = np.zeros((128, 3, C), np.float32)
        for k in range(2):
            s = k * KH + p
            keep = (~((s >= fs) & (s <= fm))) & (s < ln)
            dis = np.where(s < fs, fs - s, np.where(s > fm, s - fm, seq))
            pwk[:, k, :] = (1.0 - dis / seq) * keep
        pwk[0, 2, :] = c_slot[core]
        spanB = np.ascontiguousarray(
            span_slot[core].reshape(C, KK, 128).transpose(2, 1, 0)
        ).astype(NPF8)
        xs = en_output[core::NCORES]                     # [4, 256, 512]
        x_np = np.ascontiguousarray(
            xs.reshape(SEN, 2, KH, D).transpose(2, 1, 0, 3)).astype(NPF8)
        # memT[p, l, dj, s] = xs[l, s_glob, dj*128 + p]
        mT = xs.reshape(SEN, 2, KH, 4, 128).transpose(4, 1, 0, 3, 2)
        memT0_np = np.ascontiguousarray(mT[:, 0]).astype(NPF8)
        memT1_np = np.zeros((128, max(M1, 4)), np.float32)
        for l in range(SEN):
            if s1[l]:
                memT1_np[:, off[l]:off[l + 1]] = \
                    mT[:, 1, l, :, :s1[l]].reshape(128, 4 * s1[l])
        in_maps.append({
            "spanB": spanB, "aw": aw_np,
            "memT0": memT0_np, "memT1": memT1_np.astype(NPF8),
            "pwk": np.ascontiguousarray(pwk).astype(NPBF),
            "x": x_np,
        })
    return in_maps, assign, overflow, pls


def _host_fragment(en_output, lengths, s, e, b, att_w, att_b, tag_w, tag_b,
                   seq_len):
    """Numpy fallback for (vanishingly rare) slot-overflow fragments."""
    mem = en_output[b].astype(np.float64)
    ws = mem[s:e].sum(0)
    span = np.concatenate([mem[s], ws, mem[e - 1]])
    pos = np.arange(S)
    in_span = (pos >= s) & (pos < e)
    att_mask = in_span | (pos >= lengths[b])
    dis = np.where(pos < s, s - pos,
                   np.where(pos >= e, pos - e + 1, seq_len)).astype(np.float64)
    pwv = 1.0 - dis / seq_len
    fin = pwv[:, None] * mem
    v = span @ att_w.astype(np.float64)
    c = span @ att_b.astype(np.float64)
    sc = np.tanh(fin @ v + c)
    sc = np.where(att_mask, -1e4, sc)
    sc = sc - sc.max()
    a = np.exp(sc)
    a = a / a.sum()
    mix = a @ fin
    ms = np.concatenate([span, mix])
    lg = ms @ tag_w.astype(np.float64).T + tag_b.astype(np.float64)
    lg = lg - lg.max()
    return (lg - np.log(np.exp(lg).sum())).astype(np.float32)


def kernel(en_output, lengths, frag_b, frag_s, frag_e, att_w, att_b, tag_w,
           tag_b):
    global LAST_RESULT
    en_output = np.asarray(en_output, dtype=np.float32)
    lengths = np.asarray(lengths).astype(np.int64)
    frag_b = np.asarray(frag_b).astype(np.int64)
    frag_s = np.asarray(frag_s).astype(np.int64)
    frag_e = np.asarray(frag_e).astype(np.int64)
    att_w = np.asarray(att_w, dtype=np.float32)
    att_b = np.asarray(att_b, dtype=np.float32)
    tag_w = np.asarray(tag_w, dtype=np.float32)
    tag_b = np.asarray(tag_b, dtype=np.float32)

    # live k=1 length per sentence slot l: max over cores of the strided
    # assignment (sentence b -> core b % 8, slot l = b // 8), rounded up
    s1 = tuple(int(min(KH, max(0, (max(lengths[8 * l:8 * l + 8]) - KH + 7)
                               // 8 * 8))) for l in range(SEN))
    # per-l fragment-slot capacity: stratum max count, rounded up to 4,
    # keeping the total 16-aligned (DoubleRow moving-AP stride constraint)
    cnt = np.bincount(frag_b, minlength=B)
    gl = [max(1, int(max(cnt[8 * l:8 * l + 8]) + 3) // 4 * 4)
          for l in range(SEN)]
    gl[-1] += (-sum(gl)) % 16
    gl = tuple(gl)
    key = ("nc", s1, gl)
    if key not in _compiled:
        _compiled[key] = _build(s1, gl)
    nc = _compiled[key]

    in_maps, assign, overflow, pls = _host_prep(
        en_output, lengths, frag_b, frag_s, frag_e, att_w, att_b, tag_w,
        s1, gl)

    res = run_bass_kernel_spmd(nc, in_maps, core_ids=list(range(NCORES)),
                               trace=TRACE)
    LAST_RESULT = res

    C = sum(gl)
    SL = np.cumsum([0] + list(gl)).tolist()
    tw_mix = tag_w[:, 3 * D:]                            # [T, D]
    out = np.empty((F, T), dtype=np.float32)
    per_core = []
    for i in range(NCORES):
        dn = np.asarray(res.results[i]["out2"], np.float32)[0]  # [C]
        mx = np.asarray(res.results[i]["out3"], np.float32)     # [128, 2, D]
        mix = np.empty((C, D), np.float32)
        for l in range(SEN):
            mix[SL[l]:SL[l + 1]] = \
                mx[(l % 2) * 64:(l % 2) * 64 + gl[l], l // 2]
        per_core.append((mix @ tw_mix.T) / dn[:, None])         # [C, T]
    cores = assign[:, 0]
    slots = assign[:, 1]
    lg = np.empty((F, T), np.float32)
    for core in range(NCORES):
        sel = cores == core
        lg[sel] = per_core[core][slots[sel]]
    lg += pls + tag_b[None, :]
    m = lg.max(axis=1, keepdims=True)
    ls = lg - m
    out = ls - np.log(np.exp(ls).sum(axis=1, keepdims=True))
    seq_len = float(lengths[0])
    for i in overflow:
        out[i] = _host_fragment(en_output, lengths, int(frag_s[i]),
                                int(frag_e[i]), int(frag_b[i]), att_w, att_b,
                                tag_w, tag_b, seq_len)
    return out
